# revision 3
# baseline (speedup 1.0000x reference)
"""Trainium2 Bass kernel for CorrespondenceFeatGeneration (patch-correlation argmax flow).

Math (per image, mirrors the reference):
  fin, fref: (256, 64, 64) -> unit-normalize each pixel across channels.
  corr[p, q] = <3x3 patch of fref at p, 3x3 patch of fin at q>   (2304-dim dot)
  max_idx[q] = argmax_p corr[p, q]  (first-max tie-break)
  flow[q] = (px - qx, py - qy), zero-padded to 64x64, then 9 shifted copies.

The reference additionally row-normalizes ref patches by (||row|| + 1e-5); every
row norm is exactly sqrt(9) up to 1e-7 because pixels are unit vectors, so the
scale is uniform-positive and argmax-invariant -> skipped.

Device strategy (8 NeuronCores, SPMD):
  - Shard q (input patch positions incl. 64-grid padding): 2 images x 4 blocks
    of 1024 q-pixels. Each core computes argmax over ALL ref positions for its
    q's -> no cross-core reduction.
  - corr tile (128 q, 496 p-chunk) = sum over 9 patch offsets s and 2 channel
    halves of UinT[c, q+s] @ Uref[c, p+s]: shifted *views* of channel-major
    feature matrices -- no 9x patch materialization.
  - Precision: fp16 hi/lo split, 3 terms (hi*hi + lo*hi + hi*lo) accumulated in
    fp32 PSUM. Max error ~1e-6, well under the min top-2 corr gap (~3e-5).
    (fp32 matmul is 4 cyc/row; fp32r is tf32-like and flips argmaxes; fp16
    subnormals are exact on the PE -- probed.)
  - Argmax: VectorE max / max_index (first occurrence == reference tie-break),
    invalid p columns (px>=62) masked to -1e30 first.
Host: unit-normalize, fp16 split, pad; decode idx -> flow + 9 shifts.
"""

import numpy as np

H = W = 64
C = 256
HP = H * W          # 4096 pixel positions per image
PW_PAD = 4224       # padded ref pixel columns (>= 4096 + 130)
QW_PAD = 4352       # padded input pixel columns for windowing
QBLK = 1024         # q positions per core
QWIN = 1280         # per-core input window width (1024 + 130, padded to 1280)
NQT = 8             # q-tiles of 128 per core
NPC = 8             # p-chunks
PC = 496            # p-chunk width (8 * 496 = 3968 >= 62*64 - 1 valid max p)
PW = NPC * PC       # 3968
SHIFTS = [64 * dy + dx for dy in range(3) for dx in range(3)]

_RUNNER = None


def _build_runner():
    import sys
    if '/opt/trn_rl_repo' not in sys.path:
        sys.path.insert(0, '/opt/trn_rl_repo')
    import concourse.bass as bass  # noqa: F401
    import concourse.tile as tile
    from concourse import bacc, mybir
    from concourse.bass_utils import run_bass_kernel_spmd

    f16 = mybir.dt.float16
    f32 = mybir.dt.float32
    u32 = mybir.dt.uint32

    nc = bacc.Bacc("TRN2", target_bir_lowering=False, debug=False, num_devices=8)

    d_in = {}
    for nm in ("uin_hi", "uin_lo"):
        d_in[nm] = nc.dram_tensor(nm, [C, QWIN], f16, kind="ExternalInput").ap()
    for nm in ("uref_hi", "uref_lo"):
        d_in[nm] = nc.dram_tensor(nm, [C, PW_PAD], f16, kind="ExternalInput").ap()
    idx_d = nc.dram_tensor("idx", [128, NQT], u32, kind="ExternalOutput").ap()
    vals_d = nc.dram_tensor("vals", [128, 2 * NQT], f32, kind="ExternalOutput").ap()
    junk_d = nc.dram_tensor("junk", [1, 8], f32, kind="ExternalOutput").ap()

    TERMS = [("uin_hi", "uref_hi"), ("uin_lo", "uref_hi"), ("uin_hi", "uref_lo")]

    with tile.TileContext(nc) as tc:
        with tc.tile_pool(name="const", bufs=1) as cpool, \
             tc.tile_pool(name="corr", bufs=2) as corrpool, \
             tc.tile_pool(name="small", bufs=2) as spool, \
             tc.tile_pool(name="stage", bufs=1) as stpool, \
             tc.tile_pool(name="ps", bufs=1, space="PSUM") as ps:

            ins = {}
            for nm, w in (("uin_hi", QWIN), ("uin_lo", QWIN),
                          ("uref_hi", PW_PAD), ("uref_lo", PW_PAD)):
                for ch in range(2):
                    t = cpool.tile([128, w], f16, tag=f"{nm}{ch}")
                    nc.sync.dma_start(t[:], d_in[nm][128 * ch:128 * (ch + 1), :])
                    ins[(nm, ch)] = t

            # Wait-absorber matmuls: walrus allows only one sync wait on the
            # LDW side of a matmul, so absorb each input-DMA wait separately.
            junk_ps = ps.tile([128, PC], f32, tag="bank0")
            regions = [ins[(nm, ch)]
                       for nm in ("uin_hi", "uin_lo", "uref_hi", "uref_lo")
                       for ch in range(2)]
            for i, r in enumerate(regions):
                nc.tensor.matmul(junk_ps[:1, :8], r[:, :1], r[:, :8],
                                 start=(i == 0), stop=(i == len(regions) - 1))
            junk_sb = stpool.tile([128, 8], f32, tag="junk")
            nc.vector.tensor_copy(junk_sb[:1, :8], junk_ps[:1, :8])

            idx_stage = stpool.tile([128, NQT], u32, tag="idxs")
            vals_stage = stpool.tile([128, 2 * NQT], f32, tag="valss")

            for t in range(NQT):
                banks = [ps.tile([128, PC], f32, name=f"bank{pc}", tag=f"bank{pc}")
                         for pc in range(NPC)]
                n_steps = len(TERMS) * len(SHIFTS) * 2
                step = 0
                for (anm, bnm) in TERMS:
                    for s in SHIFTS:
                        for ch in range(2):
                            lhsT = ins[(anm, ch)][:, t * 128 + s: t * 128 + s + 128]
                            for pc in range(NPC):
                                rhs = ins[(bnm, ch)][:, pc * PC + s: pc * PC + s + PC]
                                nc.tensor.matmul(banks[pc][:], lhsT, rhs,
                                                 start=(step == 0),
                                                 stop=(step == n_steps - 1))
                            step += 1

                corr = corrpool.tile([128, PW], f32, tag="corr")
                for pc in range(NPC):
                    nc.scalar.activation(corr[:, pc * PC:(pc + 1) * PC], banks[pc][:],
                                         mybir.ActivationFunctionType.Copy)
                # mask invalid ref columns (px in {62, 63}); PW = 62*64 exactly
                mask_ap = corr[:].rearrange("p (a b) -> p a b", b=64)[:, :, 62:64]
                nc.vector.memset(mask_ap, -1e30)

                mx = spool.tile([128, 8], f32, tag="mx")
                mi = spool.tile([128, 8], u32, tag="mi")
                nc.vector.max(mx[:], corr[:])
                nc.vector.max_index(mi[:], mx[:], corr[:])
                nc.vector.tensor_copy(idx_stage[:, t:t + 1], mi[:, 0:1])
                nc.vector.tensor_copy(vals_stage[:, 2 * t:2 * t + 2], mx[:, 0:2])

            nc.sync.dma_start(idx_d[:], idx_stage[:])
            nc.sync.dma_start(vals_d[:], vals_stage[:])
            nc.sync.dma_start(junk_d[:], junk_sb[:1, :8])

    nc.compile()
    return nc, run_bass_kernel_spmd


def _unit_pixels(f):
    # f: (C, H, W) float32; unit L2 norm per pixel across channels (fp32 math)
    n = np.sqrt(np.sum(f * f, axis=0, keepdims=True, dtype=np.float32))
    return (f / np.maximum(n, np.float32(1e-12))).astype(np.float32)


def _split_f16(a):
    hi = a.astype(np.float16)
    lo = (a - hi.astype(np.float32)).astype(np.float16)
    return hi, lo


def kernel(dense_features1, dense_features2, img_ref_hr):
    global _RUNNER
    if _RUNNER is None:
        _RUNNER = _build_runner()
    nc, run_spmd = _RUNNER

    f1 = np.asarray(dense_features1, dtype=np.float32)  # input features (b,C,H,W)
    f2 = np.asarray(dense_features2, dtype=np.float32)  # ref features
    B = f1.shape[0]
    assert B == 2 and f1.shape[1:] == (C, H, W)

    in_maps = []
    per_img = []
    for b in range(B):
        fin_u = _unit_pixels(f1[b]).reshape(C, HP)
        fref_u = _unit_pixels(f2[b]).reshape(C, HP)
        uin = np.zeros((C, QW_PAD), np.float32)
        uin[:, :HP] = fin_u
        uref = np.zeros((C, PW_PAD), np.float32)
        uref[:, :HP] = fref_u
        uin_hi, uin_lo = _split_f16(uin)
        uref_hi, uref_lo = _split_f16(uref)
        per_img.append((uin_hi, uin_lo, uref_hi, uref_lo))

    for core in range(8):
        b, qblk = divmod(core, 4)
        uin_hi, uin_lo, uref_hi, uref_lo = per_img[b]
        q0 = qblk * QBLK
        in_maps.append({
            "uin_hi": np.ascontiguousarray(uin_hi[:, q0:q0 + QWIN]),
            "uin_lo": np.ascontiguousarray(uin_lo[:, q0:q0 + QWIN]),
            "uref_hi": uref_hi,
            "uref_lo": uref_lo,
        })

    results = run_spmd(nc, in_maps, list(range(8))).results

    # Decode: idx_stage[part, tile] = argmax p-flat (64-grid) for
    # q_local = tile*128 + part, global q = core_q0 + q_local.
    out = np.zeros((B, 9, H, W, 2), np.float32)
    qx_grid = np.arange(62, dtype=np.float32)[None, :]
    qy_grid = np.arange(62, dtype=np.float32)[:, None]
    for b in range(B):
        idx_full = np.zeros(HP, np.int64)
        for qblk in range(4):
            r = results[b * 4 + qblk]["idx"]  # (128, NQT) uint32
            idx_full[qblk * QBLK:(qblk + 1) * QBLK] = r.T.reshape(-1)
        idx_grid = idx_full.reshape(H, W)[:62, :62]
        py = (idx_grid // 64).astype(np.float32)
        px = (idx_grid % 64).astype(np.float32)
        flow = np.zeros((H, W, 2), np.float32)
        flow[:62, :62, 0] = px - qx_grid
        flow[:62, :62, 1] = py - qy_grid
        for k, (i, j) in enumerate([(i, j) for i in range(3) for j in range(3)]):
            out[b, k, i:, j:, :] = flow[:H - i, :W - j, :]
    return out


# revision 6
# speedup vs baseline: 1.3039x; 1.3039x over previous
"""Trainium2 Bass kernel for CorrespondenceFeatGeneration (patch-correlation argmax flow).

Math (per image, mirrors the reference):
  fin, fref: (256, 64, 64) -> unit-normalize each pixel across channels.
  corr[p, q] = <3x3 patch of fref at p, 3x3 patch of fin at q>   (2304-dim dot)
  max_idx[q] = argmax_p corr[p, q]  (first-max tie-break)
  flow[q] = (px - qx, py - qy), zero-padded to 64x64, then 9 shifted copies.

The reference additionally row-normalizes ref patches by (||row|| + 1e-5); every
row norm is exactly sqrt(9) up to 1e-7 because pixels are unit vectors, so the
scale is uniform-positive and argmax-invariant -> skipped.

Device strategy (8 NeuronCores, SPMD):
  - Shard q (input patch positions incl. 64-grid padding): 2 images x 4 blocks
    of 1024 q-pixels. Each core computes argmax over ALL ref positions for its
    q's -> no cross-core reduction.
  - corr tile (128 q, 496 p-chunk) = sum over 9 patch offsets s and 2 channel
    halves of UinT[c, q+s] @ Uref[c, p+s]: shifted *views* of channel-major
    feature matrices -- no 9x patch materialization.
  - Precision: fp16 hi/lo split, 3 terms (hi*hi + lo*hi + hi*lo) accumulated in
    fp32 PSUM. Max error ~1e-6, well under the min top-2 corr gap (~3e-5).
    (fp32 matmul is 4 cyc/row; fp32r is tf32-like and flips argmaxes; fp16
    subnormals are exact on the PE -- probed.)
  - Argmax: VectorE max / max_index (first occurrence == reference tie-break),
    invalid p columns (px>=62) masked to -1e30 first.
Host: unit-normalize, fp16 split, pad; decode idx -> flow + 9 shifts.
"""

import numpy as np

H = W = 64
C = 256
HP = H * W          # 4096 pixel positions per image
PW_PAD = 4224       # padded ref pixel columns (>= 4096 + 130)
QW_PAD = 4352       # padded input pixel columns for windowing
QBLK = 1024         # q positions per core
QWIN = 1280         # per-core input window width (1024 + 130, padded to 1280)
NQT = 8             # q-tiles of 128 per core
NPC = 8             # p-chunks
PC = 496            # p-chunk width (8 * 496 = 3968 >= 62*64 - 1 valid max p)
PW = NPC * PC       # 3968
CPW = NPC * 512     # 4096: C3 (x-patch correlation) width, 8 full PSUM banks
SHIFTS = [64 * dy + dx for dy in range(3) for dx in range(3)]

_RUNNER = None


def _build_runner():
    import sys
    if '/opt/trn_rl_repo' not in sys.path:
        sys.path.insert(0, '/opt/trn_rl_repo')
    import concourse.bass as bass  # noqa: F401
    import concourse.tile as tile
    from concourse import bacc, mybir
    from concourse.bass_utils import run_bass_kernel_spmd

    f16 = mybir.dt.float16
    f32 = mybir.dt.float32
    u32 = mybir.dt.uint32

    nc = bacc.Bacc("TRN2", target_bir_lowering=False, debug=False, num_devices=8)

    d_in = {}
    for nm in ("uin_hi", "uin_lo"):
        d_in[nm] = nc.dram_tensor(nm, [C, QWIN], f16, kind="ExternalInput").ap()
    for nm in ("uref_hi", "uref_lo"):
        d_in[nm] = nc.dram_tensor(nm, [C, PW_PAD], f16, kind="ExternalInput").ap()
    idx_d = nc.dram_tensor("idx", [128, NQT], u32, kind="ExternalOutput").ap()
    vals_d = nc.dram_tensor("vals", [128, 2 * NQT], f32, kind="ExternalOutput").ap()
    junk_d = nc.dram_tensor("junk", [1, 8], f32, kind="ExternalOutput").ap()

    TERMS = [("uin_hi", "uref_hi"), ("uin_lo", "uref_hi"), ("uin_hi", "uref_lo")]

    with tile.TileContext(nc) as tc:
        with tc.tile_pool(name="const", bufs=1) as cpool, \
             tc.tile_pool(name="corr", bufs=2) as corrpool, \
             tc.tile_pool(name="small", bufs=2) as spool, \
             tc.tile_pool(name="stage", bufs=1) as stpool, \
             tc.tile_pool(name="ps", bufs=1, space="PSUM") as ps:

            ins = {}
            for nm, w in (("uin_hi", QWIN), ("uin_lo", QWIN),
                          ("uref_hi", PW_PAD), ("uref_lo", PW_PAD)):
                for ch in range(2):
                    t = cpool.tile([128, w], f16, tag=f"{nm}{ch}")
                    nc.sync.dma_start(t[:], d_in[nm][128 * ch:128 * (ch + 1), :])
                    ins[(nm, ch)] = t

            # Wait-absorber matmuls: walrus allows only one sync wait on the
            # LDW side of a matmul, so absorb each input-DMA wait separately.
            junk_ps = ps.tile([128, PC], f32, tag="bank0")
            regions = [ins[(nm, ch)]
                       for nm in ("uin_hi", "uin_lo", "uref_hi", "uref_lo")
                       for ch in range(2)]
            for i, r in enumerate(regions):
                nc.tensor.matmul(junk_ps[:1, :8], r[:, :1], r[:, :8],
                                 start=(i == 0), stop=(i == len(regions) - 1))
            junk_sb = stpool.tile([128, 8], f32, tag="junk")
            nc.vector.tensor_copy(junk_sb[:1, :8], junk_ps[:1, :8])

            idx_stage = stpool.tile([128, NQT], u32, tag="idxs")
            vals_stage = stpool.tile([128, 2 * NQT], f32, tag="valss")

            # C3[q, p] = x-patch (1x3) correlation = sum over dx, c of
            #   Uin[c, q+dx] * Uref[c, p+dx]   (fp16 3-term split, fp32 PSUM)
            # corr[q, p] = sum over dy of C3[q + 64*dy, p + 64*dy]
            # -> PE work drops 3x vs folding all 9 offsets into the matmul;
            #    the dy-sum is 2 fp32 DVE adds (dy=2 is a whole-tile-aligned
            #    view; dy=1 needs one small partition-rebasing DMA stage).
            c3_tiles = {}
            for ct in range(NQT + 1):
                banks = [ps.tile([128, 512], f32, name=f"bank{pc}", tag=f"bank{pc}")
                         for pc in range(NPC)]
                n_steps = len(TERMS) * 3 * 2
                step = 0
                for (anm, bnm) in TERMS:
                    for dx in range(3):
                        for ch in range(2):
                            lhsT = ins[(anm, ch)][:, ct * 128 + dx: ct * 128 + dx + 128]
                            for pc in range(NPC):
                                rhs = ins[(bnm, ch)][:, pc * 512 + dx: pc * 512 + dx + 512]
                                nc.tensor.matmul(banks[pc][:], lhsT, rhs,
                                                 start=(step == 0),
                                                 stop=(step == n_steps - 1))
                            step += 1
                c3 = corrpool.tile([128, CPW], f32, name="c3", tag="c3", bufs=3)
                for pc in range(NPC):
                    nc.scalar.activation(c3[:, pc * 512:(pc + 1) * 512], banks[pc][:],
                                         mybir.ActivationFunctionType.Copy)
                c3_tiles[ct] = c3
                if ct == 0:
                    continue

                t = ct - 1
                prev, cur = c3_tiles[t], c3_tiles[ct]
                # dy=1 operand: C3 rows [t*128+64, +128) with +64 column offset
                stage = corrpool.tile([128, PW], f32, name="stage", tag="stage", bufs=2)
                nc.sync.dma_start(stage[0:64, :], prev[64:128, 64:64 + PW])
                nc.sync.dma_start(stage[64:128, :], cur[0:64, 64:64 + PW])
                corr = corrpool.tile([128, PW], f32, name="corr", tag="corr", bufs=2)
                nc.vector.tensor_add(corr[:], prev[:, 0:PW], stage[:])
                nc.vector.tensor_add(corr[:], corr[:], cur[:, 128:128 + PW])
                # mask invalid ref columns (px in {62, 63}); PW = 62*64 exactly
                mask_ap = corr[:].rearrange("p (a b) -> p a b", b=64)[:, :, 62:64]
                nc.vector.memset(mask_ap, -1e30)

                mx = spool.tile([128, 8], f32, tag="mx")
                mi = spool.tile([128, 8], u32, tag="mi")
                nc.vector.max(mx[:], corr[:])
                nc.vector.max_index(mi[:], mx[:], corr[:])
                nc.vector.tensor_copy(idx_stage[:, t:t + 1], mi[:, 0:1])
                nc.vector.tensor_copy(vals_stage[:, 2 * t:2 * t + 2], mx[:, 0:2])
                del c3_tiles[t]

            nc.sync.dma_start(idx_d[:], idx_stage[:])
            nc.sync.dma_start(vals_d[:], vals_stage[:])
            nc.sync.dma_start(junk_d[:], junk_sb[:1, :8])

    nc.compile()
    return nc, run_bass_kernel_spmd


def _unit_pixels(f):
    # f: (C, H, W) float32; unit L2 norm per pixel across channels (fp32 math)
    n = np.sqrt(np.sum(f * f, axis=0, keepdims=True, dtype=np.float32))
    return (f / np.maximum(n, np.float32(1e-12))).astype(np.float32)


def _split_f16(a):
    hi = a.astype(np.float16)
    lo = (a - hi.astype(np.float32)).astype(np.float16)
    return hi, lo


def kernel(dense_features1, dense_features2, img_ref_hr):
    global _RUNNER
    if _RUNNER is None:
        _RUNNER = _build_runner()
    nc, run_spmd = _RUNNER

    f1 = np.asarray(dense_features1, dtype=np.float32)  # input features (b,C,H,W)
    f2 = np.asarray(dense_features2, dtype=np.float32)  # ref features
    B = f1.shape[0]
    assert B == 2 and f1.shape[1:] == (C, H, W)

    in_maps = []
    per_img = []
    for b in range(B):
        fin_u = _unit_pixels(f1[b]).reshape(C, HP)
        fref_u = _unit_pixels(f2[b]).reshape(C, HP)
        uin = np.zeros((C, QW_PAD), np.float32)
        uin[:, :HP] = fin_u
        uref = np.zeros((C, PW_PAD), np.float32)
        uref[:, :HP] = fref_u
        uin_hi, uin_lo = _split_f16(uin)
        uref_hi, uref_lo = _split_f16(uref)
        per_img.append((uin_hi, uin_lo, uref_hi, uref_lo))

    for core in range(8):
        b, qblk = divmod(core, 4)
        uin_hi, uin_lo, uref_hi, uref_lo = per_img[b]
        q0 = qblk * QBLK
        in_maps.append({
            "uin_hi": np.ascontiguousarray(uin_hi[:, q0:q0 + QWIN]),
            "uin_lo": np.ascontiguousarray(uin_lo[:, q0:q0 + QWIN]),
            "uref_hi": uref_hi,
            "uref_lo": uref_lo,
        })

    results = run_spmd(nc, in_maps, list(range(8))).results

    # Decode: idx_stage[part, tile] = argmax p-flat (64-grid) for
    # q_local = tile*128 + part, global q = core_q0 + q_local.
    out = np.zeros((B, 9, H, W, 2), np.float32)
    qx_grid = np.arange(62, dtype=np.float32)[None, :]
    qy_grid = np.arange(62, dtype=np.float32)[:, None]
    for b in range(B):
        idx_full = np.zeros(HP, np.int64)
        for qblk in range(4):
            r = results[b * 4 + qblk]["idx"]  # (128, NQT) uint32
            idx_full[qblk * QBLK:(qblk + 1) * QBLK] = r.T.reshape(-1)
        idx_grid = idx_full.reshape(H, W)[:62, :62]
        py = (idx_grid // 64).astype(np.float32)
        px = (idx_grid % 64).astype(np.float32)
        flow = np.zeros((H, W, 2), np.float32)
        flow[:62, :62, 0] = px - qx_grid
        flow[:62, :62, 1] = py - qy_grid
        for k, (i, j) in enumerate([(i, j) for i in range(3) for j in range(3)]):
            out[b, k, i:, j:, :] = flow[:H - i, :W - j, :]
    return out
